# revision 10
# baseline (speedup 1.0000x reference)
"""Causal single-head attention on 8 Trainium2 NeuronCores (Bass/Tile).

Problem: x [4, 2048, 1024], W_{q,k,v} [1024, 1024] (torch Linear layout,
y = x @ W.T), causal softmax(QK^T/sqrt(D)) @ V  ->  [4, 2048, 1024] fp32.

Sharding (uniform SPMD program, per-core data only):
  core c -> batch b = c//2, key-parity h = c%2.
  Each core computes Q^T for ALL 2048 queries of its batch, and K^T/V for
  the 1024 keys with original index ≡ h (mod 2) ("virtual" keys k' with
  global key = 2k' + h). Attention is computed flash-style transposed
  (S^T[k', q] tiles), unnormalized: O_part = sum_k exp(s) V, l_part =
  sum_k exp(s). Causality over virtual keys: key 2k'+h <= query q, which
  makes every (k'-tile j, q-chunk i) block with j < i fully allowed and
  the j == i block maskable with a single slot-independent [128, 256]
  pattern (allowed iff q_l >= 2*k_l + h) -- so all 8 core programs are
  IDENTICAL and only input data differs. Host combines:
  out[b] = (O_0 + O_1) / (l_0 + l_1).

  No softmax max-subtraction: scores/32 are ~N(0, ~1.7) (randn inputs),
  exp never overflows fp32; masked entries get -1e30 pre-scale -> exp = 0.

Matmuls run in float32r (TF32-like, 1 cyc/row at N>=256; measured
~2e-4 scale-relative absmax error end-to-end on a miniature version).
"""

import os

import numpy as np

import concourse.mybir as mybir
import concourse.tile as tile
from concourse import bacc, bass_isa
from concourse.bass_utils import run_bass_kernel_spmd

F32 = mybir.dt.float32
F32R = mybir.dt.float32r

B, S, D = 4, 2048, 1024
NP = 128  # partitions
ET = D // NP  # 8 output-dim tiles (e)
DP = D // NP  # 8 contraction-dim tiles (d')
KP = S // 2  # 1024 keys per core
KT = KP // NP  # 8 key tiles
QCH = 256  # attention query-chunk width
NSLOT = S // QCH  # 8 slots
SCALE = 1.0 / 32.0  # 1/sqrt(D)
NEG = -1.0e30

_NC_CACHE = {}


def _build_nc():
    nc = bacc.Bacc(None, target_bir_lowering=False, num_devices=8)

    # host-pretiled x chunks: contiguous per partition for fat DMA descriptors
    xq = nc.dram_tensor("xq", [S // 1024, NP, DP, 512], F32R, kind="ExternalInput")
    xk = nc.dram_tensor("xk", [KP // 512, NP, DP, 512], F32R, kind="ExternalInput")
    xv = nc.dram_tensor("xv", [KT, NP, DP, NP], F32R, kind="ExternalInput")
    wqt = nc.dram_tensor("wqt", [D, D], F32R, kind="ExternalInput")  # Wq^T (d_in, e)
    wkt = nc.dram_tensor("wkt", [D, D], F32R, kind="ExternalInput")
    wvt = nc.dram_tensor("wvt", [D, D], F32R, kind="ExternalInput")
    mask = nc.dram_tensor("mask", [NP, QCH], F32, kind="ExternalInput")
    o_out = nc.dram_tensor("o", [S, D], F32, kind="ExternalOutput")
    l_out = nc.dram_tensor("l", [NSLOT, QCH], F32, kind="ExternalOutput")

    wq_r = wqt.rearrange("(t p) e -> p t e", p=NP)  # [128, 8, 1024]
    wk_r = wkt.rearrange("(t p) e -> p t e", p=NP)
    wv_r = wvt.rearrange("(t p) e -> p t e", p=NP)
    o_r = o_out.rearrange("(t p) d -> p t d", p=NP)  # [128, 16, 1024]

    with tile.TileContext(nc) as tc:
        with tc.tile_pool(name="res", bufs=1) as res:
            # residents: Q^T [e-part, e-tile, q], K^T [e-part, e-tile, k'],
            # V [k'-part, k'-tile, d]
            qt_res = res.tile([NP, ET, S], F32R)  # 64KB/p
            kt_res = res.tile([NP, ET, KP], F32R)  # 32KB/p
            v_res = res.tile([NP, KT, D], F32R)  # 32KB/p
            t_mask = res.tile([NP, QCH], F32)
            nc.sync.dma_start(t_mask[:], mask[:])

            # ---------------- projections ----------------
            with (
                tc.tile_pool(name="wp", bufs=10) as wp,
                tc.tile_pool(name="xs", bufs=2) as xs,
                tc.tile_pool(name="qst", bufs=3) as qst,
                tc.tile_pool(name="dram", bufs=1, space="DRAM") as dram,
                tc.tile_pool(name="pps", bufs=4, space="PSUM") as pps,
            ):
                qhalf_d = dram.tile([ET, NP, S // 2], F32R)
                qall_d = dram.tile([2, ET, NP, S // 2], F32R)
                # Q^T[e, q] = sum_d Wq^T[d, e]^T-stat x^T[d, q]-moving
                w_tiles = [wp.tile([NP, D], F32R, tag="w", name=f"wq{dp}") for dp in range(DP)]
                xc0 = xs.tile([NP, DP, 512], F32R, tag="xs", name="xq0")
                for dp in range(DP):
                    nc.sync.dma_start(w_tiles[dp][:], wq_r[:, dp, :])
                    nc.sync.dma_start(xc0[:, dp, :], xq[0, :, dp, :])
                for qs in range(S // 1024):
                    if qs == 0:
                        xc = xc0
                    else:
                        xc = xs.tile([NP, DP, 512], F32R, tag="xs", name=f"xq{qs}")
                        nc.sync.dma_start(xc[:], xq[qs])
                    for et in range(ET):
                        ps = pps.tile([NP, 512], F32, tag="pps", name=f"psq{qs}_{et}")
                        for dp in range(DP):
                            nc.tensor.matmul(
                                ps[:],
                                w_tiles[dp][:, et * NP : (et + 1) * NP],
                                xc[:, dp, :],
                                start=(dp == 0),
                                stop=(dp == DP - 1),
                            )
                        qsb = qst.tile([NP, 512], F32R, tag="qst", name=f"qsb{qs}_{et}")
                        nc.vector.tensor_copy(qsb[:], ps[:])
                        nc.sync.dma_start(
                            qhalf_d[et, :, qs * 512 : (qs + 1) * 512], qsb[:]
                        )
                # exchange Q^T halves within batch pairs (overlaps K/V phases)
                nc.gpsimd.collective_compute(
                    "AllGather",
                    mybir.AluOpType.bypass,
                    replica_groups=[[0, 1], [2, 3], [4, 5], [6, 7]],
                    ins=[qhalf_d[:]],
                    outs=[qall_d[:]],
                )
                for half in range(2):
                    for et in range(ET):
                        nc.sync.dma_start(
                            qt_res[:, et, half * (S // 2) : (half + 1) * (S // 2)],
                            qall_d[half, et, :, :],
                        )

                # K^T[e, k'] likewise from xkv
                w_tiles = [wp.tile([NP, D], F32R, tag="w", name=f"wk{dp}") for dp in range(DP)]
                for dp in range(DP):
                    nc.sync.dma_start(w_tiles[dp][:], wk_r[:, dp, :])
                for ks in range(KP // 512):
                    xc = xs.tile([NP, DP, 512], F32R, tag="xs", name=f"xk{ks}")
                    nc.sync.dma_start(xc[:], xk[ks])
                    for et in range(ET):
                        ps = pps.tile([NP, 512], F32, tag="pps", name=f"psk{ks}_{et}")
                        for dp in range(DP):
                            nc.tensor.matmul(
                                ps[:],
                                w_tiles[dp][:, et * NP : (et + 1) * NP],
                                xc[:, dp, :],
                                start=(dp == 0),
                                stop=(dp == DP - 1),
                            )
                        nc.vector.tensor_copy(
                            kt_res[:, et, ks * 512 : (ks + 1) * 512], ps[:]
                        )

                # V[k', d] = sum_d' xkv[d', k']-stat Wv^T[d', d]-moving
                w_tiles = [wp.tile([NP, D], F32R, tag="w", name=f"wv{dp}") for dp in range(DP)]
                for dp in range(DP):
                    nc.sync.dma_start(w_tiles[dp][:], wv_r[:, dp, :])
                for kt_i in range(KT):
                    xc = xs.tile([NP, DP, NP], F32R, tag="xs", name=f"xv{kt_i}")
                    nc.sync.dma_start(xc[:], xv[kt_i])
                    for dv in range(D // 512):
                        ps = pps.tile([NP, 512], F32, tag="pps", name=f"psv{kt_i}_{dv}")
                        for dp in range(DP):
                            nc.tensor.matmul(
                                ps[:],
                                xc[:, dp, :],
                                w_tiles[dp][:, dv * 512 : (dv + 1) * 512],
                                start=(dp == 0),
                                stop=(dp == DP - 1),
                            )
                        nc.vector.tensor_copy(
                            v_res[:, kt_i, dv * 512 : (dv + 1) * 512], ps[:]
                        )

            # ---------------- attention ----------------
            with (
                tc.tile_pool(name="pp", bufs=3) as pp,
                tc.tile_pool(name="ost", bufs=2) as ost,
                tc.tile_pool(name="sps", bufs=4, space="PSUM") as sps,
                tc.tile_pool(name="ops", bufs=1, space="PSUM") as ops,
                tc.tile_pool(name="lred", bufs=3) as lred,
                tc.tile_pool(name="lacc", bufs=2) as lacc,
            ):
                for slot in range(NSLOT):
                    o_ps = [
                        ops.tile([NP, D], F32, tag=f"o{q}", name=f"o{slot}_{q}")
                        for q in range(2)
                    ]
                    l_a = lacc.tile([1, QCH], F32, tag="lacc", name=f"lacc{slot}")
                    for j in range(slot + 1):
                        s_ps = sps.tile([NP, QCH], F32, tag="s", name=f"s{slot}_{j}")
                        for et in range(ET):
                            nc.tensor.matmul(
                                s_ps[:],
                                kt_res[:, et, j * NP : (j + 1) * NP],
                                qt_res[:, et, slot * QCH : (slot + 1) * QCH],
                                start=(et == 0),
                                stop=(et == ET - 1),
                            )
                        if j == slot:
                            nc.vector.tensor_add(s_ps[:], s_ps[:], t_mask[:])
                        p_t = pp.tile([NP, QCH], F32R, tag="p", name=f"p{slot}_{j}")
                        nc.scalar.activation(
                            out=p_t[:],
                            in_=s_ps[:],
                            func=mybir.ActivationFunctionType.Exp,
                            scale=SCALE,
                        )
                        # l partial: column sums of P via GpSimd partition reduce
                        l_red = lred.tile([NP, QCH], F32, tag="lred", name=f"lr{slot}_{j}")
                        nc.gpsimd.partition_all_reduce(
                            l_red[:], p_t[:], channels=NP, reduce_op=bass_isa.ReduceOp.add
                        )
                        if j == 0:
                            nc.vector.tensor_copy(l_a[:], l_red[0:1, :])
                        else:
                            nc.vector.tensor_add(l_a[:], l_a[:], l_red[0:1, :])
                        for q in range(2):
                            pq = p_t[:, q * NP : (q + 1) * NP]
                            for dv in range(D // 512):
                                nc.tensor.matmul(
                                    o_ps[q][:, dv * 512 : (dv + 1) * 512],
                                    pq,
                                    v_res[:, j, dv * 512 : (dv + 1) * 512],
                                    start=(j == 0),
                                    stop=(j == slot),
                                )

                    nc.sync.dma_start(l_out[slot : slot + 1, :], l_a[:])
                    for q in range(2):
                        ot = ost.tile([NP, D], F32, tag=f"ot{q}", name=f"ot{slot}_{q}")
                        nc.scalar.activation(
                            out=ot[:],
                            in_=o_ps[q][:],
                            func=mybir.ActivationFunctionType.Copy,
                        )
                        row = slot * 2 + q
                        nc.sync.dma_start(o_r[:, row, :], ot[:])
    nc.compile()
    return nc


def _get_nc():
    if "nc" not in _NC_CACHE:
        _NC_CACHE["nc"] = _build_nc()
    return _NC_CACHE["nc"]


def kernel(x, W_query, W_key, W_value):
    x = np.asarray(x, dtype=np.float32)
    wqt = np.ascontiguousarray(np.asarray(W_query, dtype=np.float32).T)
    wkt = np.ascontiguousarray(np.asarray(W_key, dtype=np.float32).T)
    wvt = np.ascontiguousarray(np.asarray(W_value, dtype=np.float32).T)

    k_l = np.arange(NP)[:, None]
    q_l = np.arange(QCH)[None, :]

    in_maps = []
    for c in range(8):
        b, h = c // 2, c % 2
        xt_b = x[b].T  # [D, S] view
        xkv_b = xt_b[:, h::2]  # [D, KP] view
        # pre-tile for contiguous-per-partition DMA chunks
        xq_half = xt_b[:, h * (S // 2) : (h + 1) * (S // 2)]
        xq_t = np.ascontiguousarray(
            xq_half.reshape(DP, NP, S // 1024, 512).transpose(2, 1, 0, 3)
        )
        xk_t = np.ascontiguousarray(
            xkv_b.reshape(DP, NP, KP // 512, 512).transpose(2, 1, 0, 3)
        )
        xv_t = np.ascontiguousarray(
            xkv_b.reshape(DP, NP, KT, NP).transpose(2, 1, 0, 3)
        )
        mask_a = np.where(q_l >= 2 * k_l + h, 0.0, NEG).astype(np.float32)
        in_maps.append(
            {
                "xq": xq_t,
                "xk": xk_t,
                "xv": xv_t,
                "wqt": wqt,
                "wkt": wkt,
                "wvt": wvt,
                "mask": mask_a,
            }
        )

    nc = _get_nc()
    res = run_bass_kernel_spmd(nc, in_maps, core_ids=list(range(8)))
    if res.exec_time_ns is not None:
        print(f"HW exec time: {res.exec_time_ns} ns")

    out = np.empty((B, S, D), dtype=np.float32)
    for b in range(B):
        o0 = res.results[2 * b]["o"]
        o1 = res.results[2 * b + 1]["o"]
        l0 = res.results[2 * b]["l"].reshape(S, 1)
        l1 = res.results[2 * b + 1]["l"].reshape(S, 1)
        out[b] = (o0 + o1) / (l0 + l1)
    return out


# revision 11
# speedup vs baseline: 1.1963x; 1.1963x over previous
"""Causal single-head attention on 8 Trainium2 NeuronCores (Bass/Tile).

Problem: x [4, 2048, 1024], W_{q,k,v} [1024, 1024] (torch Linear layout,
y = x @ W.T), causal softmax(QK^T/sqrt(D)) @ V  ->  [4, 2048, 1024] fp32.

Sharding (uniform SPMD program, per-core data only):
  core c -> batch b = c//2, key-parity h = c%2.
  Each core computes Q^T for ALL 2048 queries of its batch, and K^T/V for
  the 1024 keys with original index ≡ h (mod 2) ("virtual" keys k' with
  global key = 2k' + h). Attention is computed flash-style transposed
  (S^T[k', q] tiles), unnormalized: O_part = sum_k exp(s) V, l_part =
  sum_k exp(s). Causality over virtual keys: key 2k'+h <= query q, which
  makes every (k'-tile j, q-chunk i) block with j < i fully allowed and
  the j == i block maskable with a single slot-independent [128, 256]
  pattern (allowed iff q_l >= 2*k_l + h) -- so all 8 core programs are
  IDENTICAL and only input data differs. Host combines:
  out[b] = (O_0 + O_1) / (l_0 + l_1).

  No softmax max-subtraction: scores/32 are ~N(0, ~1.7) (randn inputs),
  exp never overflows fp32; masked entries get -1e30 pre-scale -> exp = 0.

Matmuls run in float32r (TF32-like, 1 cyc/row at N>=256; measured
~2e-4 scale-relative absmax error end-to-end on a miniature version).
"""

import os

import numpy as np

import concourse.mybir as mybir
import concourse.tile as tile
from concourse import bacc, bass_isa
from concourse.bass_utils import run_bass_kernel_spmd

F32 = mybir.dt.float32
F32R = mybir.dt.float32r

B, S, D = 4, 2048, 1024
NP = 128  # partitions
ET = D // NP  # 8 output-dim tiles (e)
DP = D // NP  # 8 contraction-dim tiles (d')
KP = S // 2  # 1024 keys per core
KT = KP // NP  # 8 key tiles
QCH = 256  # attention query-chunk width
NSLOT = S // QCH  # 8 slots
SCALE = 1.0 / 32.0  # 1/sqrt(D)
NEG = -1.0e30

_NC_CACHE = {}


def _build_nc():
    nc = bacc.Bacc(None, target_bir_lowering=False)

    # host-pretiled x chunks: contiguous per partition for fat DMA descriptors
    xq = nc.dram_tensor("xq", [S // 512, NP, DP, 512], F32R, kind="ExternalInput")
    xk = nc.dram_tensor("xk", [KP // 512, NP, DP, 512], F32R, kind="ExternalInput")
    xv = nc.dram_tensor("xv", [KT, NP, DP, NP], F32R, kind="ExternalInput")
    wqt = nc.dram_tensor("wqt", [D, D], F32R, kind="ExternalInput")  # Wq^T (d_in, e)
    wkt = nc.dram_tensor("wkt", [D, D], F32R, kind="ExternalInput")
    wvt = nc.dram_tensor("wvt", [D, D], F32R, kind="ExternalInput")
    mask = nc.dram_tensor("mask", [NP, QCH], F32, kind="ExternalInput")
    o_out = nc.dram_tensor("o", [S, D], F32, kind="ExternalOutput")
    l_out = nc.dram_tensor("l", [NSLOT, QCH], F32, kind="ExternalOutput")

    wq_r = wqt.rearrange("(t p) e -> p t e", p=NP)  # [128, 8, 1024]
    wk_r = wkt.rearrange("(t p) e -> p t e", p=NP)
    wv_r = wvt.rearrange("(t p) e -> p t e", p=NP)
    o_r = o_out.rearrange("(t p) d -> p t d", p=NP)  # [128, 16, 1024]

    with tile.TileContext(nc) as tc:
        with tc.tile_pool(name="res", bufs=1) as res:
            # residents: Q^T [e-part, e-tile, q], K^T [e-part, e-tile, k'],
            # V [k'-part, k'-tile, d]
            qt_res = res.tile([NP, ET, S], F32R)  # 64KB/p
            kt_res = res.tile([NP, ET, KP], F32R)  # 32KB/p
            v_res = res.tile([NP, KT, D], F32R)  # 32KB/p
            t_mask = res.tile([NP, QCH], F32)
            nc.sync.dma_start(t_mask[:], mask[:])

            # ---------------- projections ----------------
            with (
                tc.tile_pool(name="wp", bufs=10) as wp,
                tc.tile_pool(name="xs", bufs=2) as xs,
                tc.tile_pool(name="pps", bufs=4, space="PSUM") as pps,
            ):
                # Q^T[e, q] = sum_d Wq^T[d, e]^T-stat x^T[d, q]-moving
                w_tiles = [wp.tile([NP, D], F32R, tag="w", name=f"wq{dp}") for dp in range(DP)]
                xc0 = xs.tile([NP, DP, 512], F32R, tag="xs", name="xq0")
                for dp in range(DP):
                    nc.sync.dma_start(w_tiles[dp][:], wq_r[:, dp, :])
                    nc.sync.dma_start(xc0[:, dp, :], xq[0, :, dp, :])
                for qs in range(S // 512):
                    if qs == 0:
                        xc = xc0
                    else:
                        xc = xs.tile([NP, DP, 512], F32R, tag="xs", name=f"xq{qs}")
                        nc.sync.dma_start(xc[:], xq[qs])
                    for et in range(ET):
                        ps = pps.tile([NP, 512], F32, tag="pps", name=f"psq{qs}_{et}")
                        for dp in range(DP):
                            nc.tensor.matmul(
                                ps[:],
                                w_tiles[dp][:, et * NP : (et + 1) * NP],
                                xc[:, dp, :],
                                start=(dp == 0),
                                stop=(dp == DP - 1),
                            )
                        nc.vector.tensor_copy(
                            qt_res[:, et, qs * 512 : (qs + 1) * 512], ps[:]
                        )

                # K^T[e, k'] likewise from xkv
                w_tiles = [wp.tile([NP, D], F32R, tag="w", name=f"wk{dp}") for dp in range(DP)]
                for dp in range(DP):
                    nc.sync.dma_start(w_tiles[dp][:], wk_r[:, dp, :])
                for ks in range(KP // 512):
                    xc = xs.tile([NP, DP, 512], F32R, tag="xs", name=f"xk{ks}")
                    nc.sync.dma_start(xc[:], xk[ks])
                    for et in range(ET):
                        ps = pps.tile([NP, 512], F32, tag="pps", name=f"psk{ks}_{et}")
                        for dp in range(DP):
                            nc.tensor.matmul(
                                ps[:],
                                w_tiles[dp][:, et * NP : (et + 1) * NP],
                                xc[:, dp, :],
                                start=(dp == 0),
                                stop=(dp == DP - 1),
                            )
                        nc.vector.tensor_copy(
                            kt_res[:, et, ks * 512 : (ks + 1) * 512], ps[:]
                        )

                # V[k', d] = sum_d' xkv[d', k']-stat Wv^T[d', d]-moving
                w_tiles = [wp.tile([NP, D], F32R, tag="w", name=f"wv{dp}") for dp in range(DP)]
                for dp in range(DP):
                    nc.sync.dma_start(w_tiles[dp][:], wv_r[:, dp, :])
                for kt_i in range(KT):
                    xc = xs.tile([NP, DP, NP], F32R, tag="xs", name=f"xv{kt_i}")
                    nc.sync.dma_start(xc[:], xv[kt_i])
                    for dv in range(D // 512):
                        ps = pps.tile([NP, 512], F32, tag="pps", name=f"psv{kt_i}_{dv}")
                        for dp in range(DP):
                            nc.tensor.matmul(
                                ps[:],
                                xc[:, dp, :],
                                w_tiles[dp][:, dv * 512 : (dv + 1) * 512],
                                start=(dp == 0),
                                stop=(dp == DP - 1),
                            )
                        nc.vector.tensor_copy(
                            v_res[:, kt_i, dv * 512 : (dv + 1) * 512], ps[:]
                        )

            # ---------------- attention ----------------
            with (
                tc.tile_pool(name="pp", bufs=3) as pp,
                tc.tile_pool(name="ost", bufs=2) as ost,
                tc.tile_pool(name="sps", bufs=4, space="PSUM") as sps,
                tc.tile_pool(name="ops", bufs=1, space="PSUM") as ops,
                tc.tile_pool(name="lred", bufs=3) as lred,
                tc.tile_pool(name="lacc", bufs=2) as lacc,
            ):
                for slot in range(NSLOT):
                    o_ps = [
                        ops.tile([NP, D], F32, tag=f"o{q}", name=f"o{slot}_{q}")
                        for q in range(2)
                    ]
                    l_a = lacc.tile([1, QCH], F32, tag="lacc", name=f"lacc{slot}")
                    for j in range(slot + 1):
                        s_ps = sps.tile([NP, QCH], F32, tag="s", name=f"s{slot}_{j}")
                        for et in range(ET):
                            nc.tensor.matmul(
                                s_ps[:],
                                kt_res[:, et, j * NP : (j + 1) * NP],
                                qt_res[:, et, slot * QCH : (slot + 1) * QCH],
                                start=(et == 0),
                                stop=(et == ET - 1),
                            )
                        if j == slot:
                            nc.vector.tensor_add(s_ps[:], s_ps[:], t_mask[:])
                        p_t = pp.tile([NP, QCH], F32R, tag="p", name=f"p{slot}_{j}")
                        nc.scalar.activation(
                            out=p_t[:],
                            in_=s_ps[:],
                            func=mybir.ActivationFunctionType.Exp,
                            scale=SCALE,
                        )
                        # l partial: column sums of P via GpSimd partition reduce
                        l_red = lred.tile([NP, QCH], F32, tag="lred", name=f"lr{slot}_{j}")
                        nc.gpsimd.partition_all_reduce(
                            l_red[:], p_t[:], channels=NP, reduce_op=bass_isa.ReduceOp.add
                        )
                        if j == 0:
                            nc.vector.tensor_copy(l_a[:], l_red[0:1, :])
                        else:
                            nc.vector.tensor_add(l_a[:], l_a[:], l_red[0:1, :])
                        for q in range(2):
                            pq = p_t[:, q * NP : (q + 1) * NP]
                            for dv in range(D // 512):
                                nc.tensor.matmul(
                                    o_ps[q][:, dv * 512 : (dv + 1) * 512],
                                    pq,
                                    v_res[:, j, dv * 512 : (dv + 1) * 512],
                                    start=(j == 0),
                                    stop=(j == slot),
                                )

                    nc.sync.dma_start(l_out[slot : slot + 1, :], l_a[:])
                    for q in range(2):
                        ot = ost.tile([NP, D], F32, tag=f"ot{q}", name=f"ot{slot}_{q}")
                        nc.scalar.activation(
                            out=ot[:],
                            in_=o_ps[q][:],
                            func=mybir.ActivationFunctionType.Copy,
                        )
                        row = slot * 2 + q
                        nc.sync.dma_start(o_r[:, row, :], ot[:])
    nc.compile()
    return nc


def _get_nc():
    if "nc" not in _NC_CACHE:
        _NC_CACHE["nc"] = _build_nc()
    return _NC_CACHE["nc"]


def kernel(x, W_query, W_key, W_value):
    x = np.asarray(x, dtype=np.float32)
    wqt = np.ascontiguousarray(np.asarray(W_query, dtype=np.float32).T)
    wkt = np.ascontiguousarray(np.asarray(W_key, dtype=np.float32).T)
    wvt = np.ascontiguousarray(np.asarray(W_value, dtype=np.float32).T)

    k_l = np.arange(NP)[:, None]
    q_l = np.arange(QCH)[None, :]

    in_maps = []
    for c in range(8):
        b, h = c // 2, c % 2
        xt_b = x[b].T  # [D, S] view
        xkv_b = xt_b[:, h::2]  # [D, KP] view
        # pre-tile for contiguous-per-partition DMA chunks
        xq_t = np.ascontiguousarray(
            xt_b.reshape(DP, NP, S // 512, 512).transpose(2, 1, 0, 3)
        )
        xk_t = np.ascontiguousarray(
            xkv_b.reshape(DP, NP, KP // 512, 512).transpose(2, 1, 0, 3)
        )
        xv_t = np.ascontiguousarray(
            xkv_b.reshape(DP, NP, KT, NP).transpose(2, 1, 0, 3)
        )
        mask_a = np.where(q_l >= 2 * k_l + h, 0.0, NEG).astype(np.float32)
        in_maps.append(
            {
                "xq": xq_t,
                "xk": xk_t,
                "xv": xv_t,
                "wqt": wqt,
                "wkt": wkt,
                "wvt": wvt,
                "mask": mask_a,
            }
        )

    nc = _get_nc()
    res = run_bass_kernel_spmd(nc, in_maps, core_ids=list(range(8)))
    if res.exec_time_ns is not None:
        print(f"HW exec time: {res.exec_time_ns} ns")

    out = np.empty((B, S, D), dtype=np.float32)
    for b in range(B):
        o0 = res.results[2 * b]["o"]
        o1 = res.results[2 * b + 1]["o"]
        l0 = res.results[2 * b]["l"].reshape(S, 1)
        l1 = res.results[2 * b + 1]["l"].reshape(S, 1)
        out[b] = (o0 + o1) / (l0 + l1)
    return out
